# revision 43
# baseline (speedup 1.0000x reference)
"""Causal self-attention (B=8, T=1024, C=768, H=8 heads) for 8 TRN2 NeuronCores.

Strategy: pure data parallelism — one batch element per core. Each core runs an
identical Bass/Tile program computing the full attention block for its batch
element; weights are replicated. No collectives.

Per-core pipeline (all matmuls in fp32r — full-rate TF32-class PE mode):
  1. x [T,C] -> x^T [C,T] via PE transposes (contraction dim must be on
     partitions).
  2. v = x @ W_v + b_v in [token, feat] layout, stored per 128-token block as
     v_aug [128, 8*97]: per head 96 value columns plus a ones column (the ones
     column makes the P@V matmul also produce the softmax denominator).
  3. Per head h (interleaved so PE work overlaps the ACT exp work of previous
     heads): q^T, k^T = (x @ W_{q,k} + b)^T in [d, token] layout ([96, 1024]),
     then S^T[tk,q] = k^T.T @ q^T per 128-key block over the causally-valid
     query range; P = exp(S*scale) with the diagonal triangle masked by a DVE
     multiply; y_aug^T = sum_tk v_aug^T P (row 96 = softmax denominator since
     P's key-axis is the partition axis); y^T = y_aug^T[0:96] *
     broadcast(Exp(-Ln(denominator))) (Ln+Exp share one ACT table, unlike
     Reciprocal; the broadcast is a 0-stride DMA).
  4. out = y @ W_proj + b_proj with the feature-packed y^T as lhsT.
"""
import sys

sys.path.insert(0, "/opt/trn_rl_repo")

import numpy as np

T, C, H, D = 1024, 768, 8, 96
C3 = 3 * C
P = 128
NT = T // P   # 8 token blocks
NCB = C // P  # 6 feature blocks
DA = D + 1    # 97: head dim + denominator column

_CACHE = {}


def _build():
    import concourse.bacc as bacc
    import concourse.mybir as mybir
    import concourse.tile as tile
    from concourse.masks import make_identity

    F32 = mybir.dt.float32
    F32R = mybir.dt.float32r
    Exp = mybir.ActivationFunctionType.Exp
    Ln = mybir.ActivationFunctionType.Ln
    is_ge = mybir.AluOpType.is_ge
    SCALE = 1.0 / float(np.sqrt(D))

    nc = bacc.Bacc("TRN2", target_bir_lowering=False, debug=False, num_devices=8)
    x_d = nc.dram_tensor("x", [T, C], F32, kind="ExternalInput").ap()
    wa_d = nc.dram_tensor("W_attn", [C, C3], F32, kind="ExternalInput").ap()
    ba_d = nc.dram_tensor("b_attn", [C3], F32, kind="ExternalInput").ap()
    wp_d = nc.dram_tensor("W_proj", [C, C], F32, kind="ExternalInput").ap()
    bp_d = nc.dram_tensor("b_proj", [C], F32, kind="ExternalInput").ap()
    out_d = nc.dram_tensor("out", [T, C], F32, kind="ExternalOutput").ap()

    with tile.TileContext(nc) as tc:
        with tc.tile_pool(name="const", bufs=1) as const_p, \
             tc.tile_pool(name="vp", bufs=1) as v_p, \
             tc.tile_pool(name="qkt", bufs=5) as qk_p, \
             tc.tile_pool(name="yt", bufs=1) as yT_p, \
             tc.tile_pool(name="sm", bufs=2) as sm_p, \
             tc.tile_pool(name="ob", bufs=1) as o_p, \
             tc.tile_pool(name="pp", bufs=4) as p_p, \
             tc.tile_pool(name="ps", bufs=1, space="PSUM") as ps:
            # pack small f32 constants into one 4KB slot:
            #   cols 0:128 identity, 128:256 triangle mask, 256:264 ones8,
            #   264:280 per-head q/k bias columns
            constF = const_p.tile([P, 280], F32, name="constF")
            ident = constF[:, 0:P]
            tri = constF[:, P:2 * P]
            ones8_f = constF[:, 2 * P:2 * P + H]
            b_qk = constF[0:D, 2 * P + H:2 * P + H + 16]
            make_identity(nc, ident)
            # lower-left triangle mask: tri[tk, u] = 1.0 iff u >= tk
            nc.gpsimd.memset(tri, 1.0)
            nc.gpsimd.affine_select(
                out=tri, in_=tri, compare_op=is_ge, fill=0.0,
                base=0, pattern=[[1, P]], channel_multiplier=-1)
            nc.vector.memset(ones8_f, 1.0)
            # col h = q-head h bias, col 8+h = k-head h bias
            nc.sync.dma_start(b_qk, ba_d.rearrange("(a b) -> b a", b=D)[:, 0:16])
            # bias rows broadcast to all partitions via 0-stride DMA
            bv_bc = const_p.tile([P, C], F32, name="bv_bc")
            nc.sync.dma_start(
                bv_bc[:],
                ba_d.unsqueeze(0)[:, 2 * C:3 * C].partition_broadcast(P).squeeze(1))
            vA = [v_p.tile([P, DA * H], F32R, name=f"vA{t}") for t in range(NT)]
            yTp = [yT_p.tile([P, T], F32R, name=f"yTp{cb}") for cb in range(NCB)]
            # DRAM staging for the reciprocal rows (SBUF->SBUF DMA cannot do
            # 0-stride broadcast, DRAM->SBUF can)
            rc_dram = nc.dram_tensor("rc_stage", [2 * H, 512], F32,
                                     kind="Internal").ap()

            with tc.tile_pool(name="xT", bufs=1) as xT_p, \
                 tc.tile_pool(name="wqk", bufs=1) as wqk_p:
                xT = [xT_p.tile([P, T], F32R, name=f"xT{cb}") for cb in range(NCB)]

                # ---- x^T transposes + v projection (scoped W_v / x loads) ----
                with tc.tile_pool(name="xl", bufs=4) as x_p, \
                     tc.tile_pool(name="wv", bufs=1) as wv_p:
                    for jt in range(2):
                        x_ts = []
                        for tb in range(4 * jt, 4 * jt + 4):
                            x_t = x_p.tile([P, C], F32, name="x_t")
                            nc.sync.dma_start(x_t[:], x_d[tb * P:(tb + 1) * P, :])
                            x_ts.append(x_t)
                        for cb in range(NCB):
                            tr_ps = ps.tile([P, 512], F32, name="tr_ps", tag="big", bufs=3)
                            for k in range(4):
                                nc.tensor.transpose(tr_ps[:, k * P:(k + 1) * P],
                                                    x_ts[k][:, cb * P:(cb + 1) * P],
                                                    ident)
                            nc.vector.tensor_copy(xT[cb][:, jt * 512:(jt + 1) * 512],
                                                  tr_ps[:])

                    wv = []
                    for cb in range(NCB):
                        w = wv_p.tile([P, C], F32R, name=f"wv{cb}")
                        nc.sync.dma_start(w[:], wa_d[cb * P:(cb + 1) * P,
                                                     2 * C:3 * C].bitcast(F32R))
                        wv.append(w)
                    for tb in range(NT):
                        v_ps = ps.tile([P, C], F32, name="v_ps", tag="big", bufs=3)
                        for cb in range(NCB):
                            lhsT = xT[cb][:, tb * P:(tb + 1) * P]
                            nc.tensor.matmul(v_ps[:, 0:512], lhsT, wv[cb][:, 0:512],
                                             start=(cb == 0), stop=(cb == NCB - 1))
                            nc.tensor.matmul(v_ps[:, 512:C], lhsT, wv[cb][:, 512:C],
                                             start=(cb == 0), stop=(cb == NCB - 1))
                        for h in range(H):
                            nc.vector.tensor_add(vA[tb][:, DA * h:DA * h + D],
                                                 v_ps[:, D * h:D * h + D],
                                                 bv_bc[:, D * h:D * h + D])
                        # ones columns at local col 96 of each head's group
                        nc.vector.tensor_copy(vA[tb][:, D::DA], ones8_f)

                # ---- per-head: q/k projection + attention, interleaved ----
                wqk = []
                for cb in range(NCB):
                    w = wqk_p.tile([P, 2 * C], F32R, name=f"wqk{cb}")
                    nc.sync.dma_start(w[:], wa_d[cb * P:(cb + 1) * P,
                                                 0:2 * C].bitcast(F32R))
                    wqk.append(w)

                def emit_tail(h, y_sbs):
                    # softmax denominator tail for head h — emitted one head
                    # late so its ACT work (Ln/Exp) doesn't preempt the next
                    # head's critical exp chain
                    for half, y_sb in ((0, y_sbs[0]), (1, y_sbs[1])):
                        q_sl = slice(half * 512, (half + 1) * 512)
                        lnrc = sm_p.tile([1, 1024], F32, name="lnrc", tag="lnrc", bufs=2)
                        ln_s = lnrc[:, 0:512]
                        rc_f = lnrc[:, 512:1024]
                        nc.scalar.activation(ln_s, y_sb[D:DA, :], Ln)
                        nc.scalar.activation(rc_f, ln_s, Exp, scale=-1.0)
                        row = rc_dram[2 * h + half:2 * h + half + 1, :]
                        nc.sync.dma_start(row, rc_f)
                        bc_sb = sm_p.tile([D, 512], F32, name="bc_sb", tag="bcsb", bufs=2)
                        nc.sync.dma_start(bc_sb[:],
                                          row.partition_broadcast(D).squeeze(1))
                        y_n = sm_p.tile([D, 512], F32R, name="y_n", tag="yn", bufs=2)
                        nc.vector.tensor_mul(y_n[:], y_sb[0:D, :], bc_sb[:])
                        # scatter head rows into the feature-packed yT tiles
                        # (partition shift -> must go through DMA)
                        f0 = D * h
                        while f0 < D * (h + 1):
                            cb2, b0 = f0 // P, f0 % P
                            seg = min(P - b0, D * (h + 1) - f0)
                            nc.sync.dma_start(
                                yTp[cb2][b0:b0 + seg, q_sl],
                                y_n[f0 - D * h:f0 - D * h + seg, :])
                            f0 += seg

                pending = None
                for h in range(H):
                    qT = qk_p.tile([D, T], F32R, name="qT", tag="qkt")
                    kT = qk_p.tile([D, T], F32R, name="kT", tag="qkt")
                    for dst, off, bcol in ((qT, D * h, b_qk[:, h:h + 1]),
                                           (kT, C + D * h, b_qk[:, 8 + h:9 + h])):
                        qk_ps = ps.tile([D, T], F32, name="qk_ps", tag="big", bufs=3)
                        for jt in range(2):
                            sl = slice(jt * 512, (jt + 1) * 512)
                            for cb in range(NCB):
                                nc.tensor.matmul(qk_ps[:, sl],
                                                 wqk[cb][:, off:off + D],
                                                 xT[cb][:, sl],
                                                 start=(cb == 0), stop=(cb == NCB - 1))
                            # quarter-width copybacks overlap the remaining
                            # matmuls, so the first S block starts sooner
                            for qq in range(2):
                                q_sl2 = slice(jt * 512 + qq * 256,
                                              jt * 512 + (qq + 1) * 256)
                                nc.vector.tensor_scalar_add(dst[:, q_sl2],
                                                            qk_ps[:, q_sl2], bcol)

                    ptiles = []
                    for ib in range(NT):
                        q0 = P * ib
                        # S^T block over the valid query range [q0, 1024),
                        # split at the 512 PSUM-bank boundary
                        s_ps = ps.tile([P, T], F32, name="s_ps", tag="big", bufs=3)
                        kblk = kT[:, ib * P:(ib + 1) * P]
                        if q0 < 512:
                            nc.tensor.matmul(s_ps[:, q0:512], kblk,
                                             qT[:, q0:512], start=True, stop=True)
                        r0 = max(q0, 512)
                        nc.tensor.matmul(s_ps[:, r0:T], kblk,
                                         qT[:, r0:T], start=True, stop=True)
                        p_t = p_p.tile([P, T], F32R, name="p_t")
                        nc.scalar.activation(p_t[:, q0:T], s_ps[:, q0:T],
                                             Exp, scale=SCALE)
                        # zero the upper triangle of the diagonal 128-col block
                        # (DVE mask-mul: gpsimd's sem wake is too slow here)
                        nc.vector.tensor_mul(p_t[:, q0:q0 + P],
                                             p_t[:, q0:q0 + P], tri)
                        ptiles.append(p_t)
                    # P@V with causal N-restriction; two bank-halves of q,
                    # each its own accumulation group
                    y_l = ps.tile([DA, 512], F32, name="y_l", tag="yps", bufs=2)
                    y_r = ps.tile([DA, 512], F32, name="y_r", tag="yps", bufs=2)
                    for ib in range(NT):
                        q0 = P * ib
                        va = vA[ib][:, DA * h:DA * h + DA]
                        if q0 < 512:
                            nc.tensor.matmul(y_l[:, q0:512], va,
                                             ptiles[ib][:, q0:512],
                                             start=(ib == 0), stop=(ib == 3))
                            nc.tensor.matmul(y_r[:], va, ptiles[ib][:, 512:T],
                                             start=(ib == 0), stop=False)
                        else:
                            nc.tensor.matmul(y_r[:, q0 - 512:512], va,
                                             ptiles[ib][:, q0:T],
                                             start=False, stop=(ib == NT - 1))
                    # stage y to SBUF (frees the PSUM slot quickly); the
                    # denominator tail is emitted one head later
                    y_sbs = []
                    for y_ps in (y_l, y_r):
                        y_sb = sm_p.tile([DA, 512], F32, name="y_sb", tag="ysb", bufs=4)
                        nc.vector.tensor_copy(y_sb[:], y_ps[:])
                        y_sbs.append(y_sb)
                    if pending is not None:
                        emit_tail(*pending)
                    pending = (h, y_sbs)
                if pending is not None:
                    emit_tail(*pending)

            # ---------------- projection ----------------
            with tc.tile_pool(name="wp", bufs=1) as wp_p:
                bp_bc = wp_p.tile([P, C], F32, name="bp_bc", tag="bpbc", bufs=1)
                nc.sync.dma_start(
                    bp_bc[:], bp_d.unsqueeze(0).partition_broadcast(P).squeeze(1))
                wp = []
                for cb in range(NCB):
                    w = wp_p.tile([P, C], F32R, name=f"wp{cb}")
                    nc.sync.dma_start(w[:], wp_d[cb * P:(cb + 1) * P, :].bitcast(F32R))
                    wp.append(w)
                for tb in range(NT):
                    o_ps = ps.tile([P, C], F32, name="o_ps", tag="big", bufs=3)
                    for cb in range(NCB):
                        lhsT = yTp[cb][:, tb * P:(tb + 1) * P]
                        nc.tensor.matmul(o_ps[:, 0:512], lhsT, wp[cb][:, 0:512],
                                         start=(cb == 0), stop=(cb == NCB - 1))
                        nc.tensor.matmul(o_ps[:, 512:C], lhsT, wp[cb][:, 512:C],
                                         start=(cb == 0), stop=(cb == NCB - 1))
                    o_sb = o_p.tile([P, C], F32, name="o_sb")
                    nc.vector.tensor_add(o_sb[:], o_ps[:], bp_bc[:])
                    nc.sync.dma_start(out_d[tb * P:(tb + 1) * P, :], o_sb[:])

    # The act-table-load pass assigns each activation the first table set
    # containing its function, which puts Exp in exp_and_others and Ln in
    # natural_log — a 1.3us table reload on every switch. All our functions
    # (Exp, Ln, Identity) live together in natural_log_exp_and_others, so
    # hide the other sets (keeping dict order — act_func_set_id is positional)
    # during this build.
    import concourse.hw_specs as hw_specs
    orig_tables = hw_specs.get_activation_tables

    def _tables(arch, *a, **kw):
        tabs = orig_tables(arch, *a, **kw)
        pref = "natural_log_exp_and_others"
        if pref not in tabs:
            return tabs
        return {k: (v if k == pref else type(v)()) for k, v in tabs.items()}

    import concourse.bacc as bacc_mod
    hw_specs.get_activation_tables = _tables
    bacc_orig = getattr(bacc_mod, "get_activation_tables", None)
    try:
        if bacc_orig is not None:
            bacc_mod.get_activation_tables = _tables
        nc.compile()
    finally:
        hw_specs.get_activation_tables = orig_tables
        if bacc_orig is not None:
            bacc_mod.get_activation_tables = bacc_orig
    return nc


def run(inputs, trace=False):
    import concourse.bass_utils as bass_utils

    nc = _CACHE.get("nc")
    if nc is None:
        nc = _CACHE["nc"] = _build()

    x = np.ascontiguousarray(inputs["x"], dtype=np.float32)
    wa = np.ascontiguousarray(inputs["W_attn"], dtype=np.float32)
    ba = np.ascontiguousarray(inputs["b_attn"], dtype=np.float32)
    wp = np.ascontiguousarray(inputs["W_proj"], dtype=np.float32)
    bp = np.ascontiguousarray(inputs["b_proj"], dtype=np.float32)
    B = x.shape[0]
    in_maps = [
        {"x": np.ascontiguousarray(x[b]), "W_attn": wa, "b_attn": ba,
         "W_proj": wp, "b_proj": bp}
        for b in range(B)
    ]
    res = bass_utils.run_bass_kernel_spmd(
        nc, in_maps, core_ids=list(range(B)), trace=trace)
    out = np.stack([r["out"] for r in res.results], axis=0)
    return out, res


def kernel(**inputs):
    out, _ = run(inputs, trace=False)
    return out


# revision 44
# speedup vs baseline: 1.0471x; 1.0471x over previous
"""Causal self-attention (B=8, T=1024, C=768, H=8 heads) for 8 TRN2 NeuronCores.

Strategy: pure data parallelism — one batch element per core. Each core runs an
identical Bass/Tile program computing the full attention block for its batch
element; weights are replicated. No collectives.

Per-core pipeline (all matmuls in fp32r — full-rate TF32-class PE mode):
  1. x [T,C] -> x^T [C,T] via PE transposes (contraction dim must be on
     partitions).
  2. v = x @ W_v + b_v in [token, feat] layout, stored per 128-token block as
     v_aug [128, 8*97]: per head 96 value columns plus a ones column (the ones
     column makes the P@V matmul also produce the softmax denominator).
  3. Per head h (interleaved so PE work overlaps the ACT exp work of previous
     heads): q^T, k^T = (x @ W_{q,k} + b)^T in [d, token] layout ([96, 1024]),
     then S^T[tk,q] = k^T.T @ q^T per 128-key block over the causally-valid
     query range; P = exp(S*scale) with the diagonal triangle masked by a DVE
     multiply; y_aug^T = sum_tk v_aug^T P (row 96 = softmax denominator since
     P's key-axis is the partition axis); y^T = y_aug^T[0:96] *
     broadcast(Exp(-Ln(denominator))) (Ln+Exp share one ACT table, unlike
     Reciprocal; the broadcast is a 0-stride DMA).
  4. out = y @ W_proj + b_proj with the feature-packed y^T as lhsT.
"""
import sys

sys.path.insert(0, "/opt/trn_rl_repo")

import numpy as np

T, C, H, D = 1024, 768, 8, 96
C3 = 3 * C
P = 128
NT = T // P   # 8 token blocks
NCB = C // P  # 6 feature blocks
DA = D + 1    # 97: head dim + denominator column

_CACHE = {}


def _build():
    import concourse.bacc as bacc
    import concourse.mybir as mybir
    import concourse.tile as tile
    from concourse.masks import make_identity

    F32 = mybir.dt.float32
    F32R = mybir.dt.float32r
    Exp = mybir.ActivationFunctionType.Exp
    Ln = mybir.ActivationFunctionType.Ln
    is_ge = mybir.AluOpType.is_ge
    SCALE = 1.0 / float(np.sqrt(D))

    nc = bacc.Bacc("TRN2", target_bir_lowering=False, debug=False, num_devices=8)
    x_d = nc.dram_tensor("x", [T, C], F32, kind="ExternalInput").ap()
    wa_d = nc.dram_tensor("W_attn", [C, C3], F32, kind="ExternalInput").ap()
    ba_d = nc.dram_tensor("b_attn", [C3], F32, kind="ExternalInput").ap()
    wp_d = nc.dram_tensor("W_proj", [C, C], F32, kind="ExternalInput").ap()
    bp_d = nc.dram_tensor("b_proj", [C], F32, kind="ExternalInput").ap()
    out_d = nc.dram_tensor("out", [T, C], F32, kind="ExternalOutput").ap()

    with tile.TileContext(nc) as tc:
        with tc.tile_pool(name="const", bufs=1) as const_p, \
             tc.tile_pool(name="vp", bufs=1) as v_p, \
             tc.tile_pool(name="qkt", bufs=5) as qk_p, \
             tc.tile_pool(name="yt", bufs=1) as yT_p, \
             tc.tile_pool(name="sm", bufs=2) as sm_p, \
             tc.tile_pool(name="ob", bufs=1) as o_p, \
             tc.tile_pool(name="pp", bufs=4) as p_p, \
             tc.tile_pool(name="ps", bufs=1, space="PSUM") as ps:
            # pack small f32 constants into one 4KB slot:
            #   cols 0:128 identity, 128:256 triangle mask, 256:264 ones8,
            #   264:280 per-head q/k bias columns
            constF = const_p.tile([P, 280], F32, name="constF")
            ident = constF[:, 0:P]
            tri = constF[:, P:2 * P]
            ones8_f = constF[:, 2 * P:2 * P + H]
            b_qk = constF[0:D, 2 * P + H:2 * P + H + 16]
            make_identity(nc, ident)
            # lower-left triangle mask: tri[tk, u] = 1.0 iff u >= tk
            nc.gpsimd.memset(tri, 1.0)
            nc.gpsimd.affine_select(
                out=tri, in_=tri, compare_op=is_ge, fill=0.0,
                base=0, pattern=[[1, P]], channel_multiplier=-1)
            nc.vector.memset(ones8_f, 1.0)
            # col h = q-head h bias, col 8+h = k-head h bias
            nc.sync.dma_start(b_qk, ba_d.rearrange("(a b) -> b a", b=D)[:, 0:16])
            # bias rows broadcast to all partitions via 0-stride DMA
            bv_bc = const_p.tile([P, C], F32, name="bv_bc")
            nc.sync.dma_start(
                bv_bc[:],
                ba_d.unsqueeze(0)[:, 2 * C:3 * C].partition_broadcast(P).squeeze(1))
            vA = [v_p.tile([P, DA * H], F32R, name=f"vA{t}") for t in range(NT)]
            yTp = [yT_p.tile([P, T], F32R, name=f"yTp{cb}") for cb in range(NCB)]
            # DRAM staging for the reciprocal rows (SBUF->SBUF DMA cannot do
            # 0-stride broadcast, DRAM->SBUF can)
            rc_dram = nc.dram_tensor("rc_stage", [2 * H, 512], F32,
                                     kind="Internal").ap()

            with tc.tile_pool(name="xT", bufs=1) as xT_p, \
                 tc.tile_pool(name="wqk", bufs=1) as wqk_p:
                xT = [xT_p.tile([P, T], F32R, name=f"xT{cb}") for cb in range(NCB)]

                # ---- x^T transposes + v projection (scoped W_v / x loads) ----
                with tc.tile_pool(name="xl", bufs=4) as x_p, \
                     tc.tile_pool(name="wv", bufs=1) as wv_p:
                    for jt in range(2):
                        x_ts = []
                        for tb in range(4 * jt, 4 * jt + 4):
                            x_t = x_p.tile([P, C], F32, name="x_t")
                            nc.sync.dma_start(x_t[:], x_d[tb * P:(tb + 1) * P, :])
                            x_ts.append(x_t)
                        for cb in range(NCB):
                            tr_ps = ps.tile([P, 512], F32, name="tr_ps", tag="big", bufs=3)
                            for k in range(4):
                                nc.tensor.transpose(tr_ps[:, k * P:(k + 1) * P],
                                                    x_ts[k][:, cb * P:(cb + 1) * P],
                                                    ident)
                            nc.vector.tensor_copy(xT[cb][:, jt * 512:(jt + 1) * 512],
                                                  tr_ps[:])

                    wv = []
                    for cb in range(NCB):
                        w = wv_p.tile([P, C], F32R, name=f"wv{cb}")
                        nc.sync.dma_start(w[:], wa_d[cb * P:(cb + 1) * P,
                                                     2 * C:3 * C].bitcast(F32R))
                        wv.append(w)
                    for tb in range(NT):
                        v_ps = ps.tile([P, C], F32, name="v_ps", tag="big", bufs=3)
                        for cb in range(NCB):
                            lhsT = xT[cb][:, tb * P:(tb + 1) * P]
                            nc.tensor.matmul(v_ps[:, 0:512], lhsT, wv[cb][:, 0:512],
                                             start=(cb == 0), stop=(cb == NCB - 1))
                            nc.tensor.matmul(v_ps[:, 512:C], lhsT, wv[cb][:, 512:C],
                                             start=(cb == 0), stop=(cb == NCB - 1))
                        for h in range(H):
                            nc.vector.tensor_add(vA[tb][:, DA * h:DA * h + D],
                                                 v_ps[:, D * h:D * h + D],
                                                 bv_bc[:, D * h:D * h + D])
                        # ones columns at local col 96 of each head's group
                        nc.vector.tensor_copy(vA[tb][:, D::DA], ones8_f)

                # ---- per-head: q/k projection + attention, interleaved ----
                wqk = []
                for cb in range(NCB):
                    w = wqk_p.tile([P, 2 * C], F32R, name=f"wqk{cb}")
                    nc.sync.dma_start(w[:], wa_d[cb * P:(cb + 1) * P,
                                                 0:2 * C].bitcast(F32R))
                    wqk.append(w)

                def emit_tail(h, y_sbs):
                    # softmax denominator tail for head h — emitted one head
                    # late so its ACT work (Ln/Exp) doesn't preempt the next
                    # head's critical exp chain
                    for half, y_sb in ((0, y_sbs[0]), (1, y_sbs[1])):
                        q_sl = slice(half * 512, (half + 1) * 512)
                        lnrc = sm_p.tile([1, 1024], F32, name="lnrc", tag="lnrc", bufs=2)
                        ln_s = lnrc[:, 0:512]
                        rc_f = lnrc[:, 512:1024]
                        nc.scalar.activation(ln_s, y_sb[D:DA, :], Ln)
                        nc.scalar.activation(rc_f, ln_s, Exp, scale=-1.0)
                        row = rc_dram[2 * h + half:2 * h + half + 1, :]
                        nc.sync.dma_start(row, rc_f)
                        bc_sb = sm_p.tile([D, 512], F32, name="bc_sb", tag="bcsb", bufs=2)
                        nc.sync.dma_start(bc_sb[:],
                                          row.partition_broadcast(D).squeeze(1))
                        y_n = sm_p.tile([D, 512], F32R, name="y_n", tag="yn", bufs=2)
                        nc.vector.tensor_mul(y_n[:], y_sb[0:D, :], bc_sb[:])
                        # scatter head rows into the feature-packed yT tiles
                        # (partition shift -> must go through DMA)
                        f0 = D * h
                        while f0 < D * (h + 1):
                            cb2, b0 = f0 // P, f0 % P
                            seg = min(P - b0, D * (h + 1) - f0)
                            nc.sync.dma_start(
                                yTp[cb2][b0:b0 + seg, q_sl],
                                y_n[f0 - D * h:f0 - D * h + seg, :])
                            f0 += seg

                pending = None
                for h in range(H):
                    qT = qk_p.tile([D, T], F32R, name="qT", tag="qkt")
                    kT = qk_p.tile([D, T], F32R, name="kT", tag="qkt")
                    for dst, off, bcol in ((qT, D * h, b_qk[:, h:h + 1]),
                                           (kT, C + D * h, b_qk[:, 8 + h:9 + h])):
                        qk_ps = ps.tile([D, T], F32, name="qk_ps", tag="big", bufs=3)
                        for jt in range(2):
                            sl = slice(jt * 512, (jt + 1) * 512)
                            for cb in range(NCB):
                                nc.tensor.matmul(qk_ps[:, sl],
                                                 wqk[cb][:, off:off + D],
                                                 xT[cb][:, sl],
                                                 start=(cb == 0), stop=(cb == NCB - 1))
                            # per-half copyback: overlaps the other half's
                            # matmuls, so the first S block never waits on a
                            # full-width DVE copy
                            nc.vector.tensor_scalar_add(dst[:, sl], qk_ps[:, sl],
                                                        bcol)

                    ptiles = []
                    for ib in range(NT):
                        q0 = P * ib
                        # S^T block over the valid query range [q0, 1024),
                        # split at the 512 PSUM-bank boundary
                        s_ps = ps.tile([P, T], F32, name="s_ps", tag="big", bufs=3)
                        kblk = kT[:, ib * P:(ib + 1) * P]
                        if q0 < 512:
                            nc.tensor.matmul(s_ps[:, q0:512], kblk,
                                             qT[:, q0:512], start=True, stop=True)
                        r0 = max(q0, 512)
                        nc.tensor.matmul(s_ps[:, r0:T], kblk,
                                         qT[:, r0:T], start=True, stop=True)
                        p_t = p_p.tile([P, T], F32R, name="p_t")
                        nc.scalar.activation(p_t[:, q0:T], s_ps[:, q0:T],
                                             Exp, scale=SCALE)
                        # zero the upper triangle of the diagonal 128-col block
                        # (DVE mask-mul: gpsimd's sem wake is too slow here)
                        nc.vector.tensor_mul(p_t[:, q0:q0 + P],
                                             p_t[:, q0:q0 + P], tri)
                        ptiles.append(p_t)
                    # P@V with causal N-restriction; two bank-halves of q,
                    # each its own accumulation group
                    y_l = ps.tile([DA, 512], F32, name="y_l", tag="yps", bufs=2)
                    y_r = ps.tile([DA, 512], F32, name="y_r", tag="yps", bufs=2)
                    for ib in range(NT):
                        q0 = P * ib
                        va = vA[ib][:, DA * h:DA * h + DA]
                        if q0 < 512:
                            nc.tensor.matmul(y_l[:, q0:512], va,
                                             ptiles[ib][:, q0:512],
                                             start=(ib == 0), stop=(ib == 3))
                            nc.tensor.matmul(y_r[:], va, ptiles[ib][:, 512:T],
                                             start=(ib == 0), stop=False)
                        else:
                            nc.tensor.matmul(y_r[:, q0 - 512:512], va,
                                             ptiles[ib][:, q0:T],
                                             start=False, stop=(ib == NT - 1))
                    # stage y to SBUF (frees the PSUM slot quickly); the
                    # denominator tail is emitted one head later
                    y_sbs = []
                    for y_ps in (y_l, y_r):
                        y_sb = sm_p.tile([DA, 512], F32, name="y_sb", tag="ysb", bufs=4)
                        nc.vector.tensor_copy(y_sb[:], y_ps[:])
                        y_sbs.append(y_sb)
                    if pending is not None:
                        emit_tail(*pending)
                    pending = (h, y_sbs)
                if pending is not None:
                    emit_tail(*pending)

            # ---------------- projection ----------------
            with tc.tile_pool(name="wp", bufs=1) as wp_p:
                bp_bc = wp_p.tile([P, C], F32, name="bp_bc", tag="bpbc", bufs=1)
                nc.sync.dma_start(
                    bp_bc[:], bp_d.unsqueeze(0).partition_broadcast(P).squeeze(1))
                wp = []
                for cb in range(NCB):
                    w = wp_p.tile([P, C], F32R, name=f"wp{cb}")
                    nc.sync.dma_start(w[:], wp_d[cb * P:(cb + 1) * P, :].bitcast(F32R))
                    wp.append(w)
                for tb in range(NT):
                    o_ps = ps.tile([P, C], F32, name="o_ps", tag="big", bufs=3)
                    for cb in range(NCB):
                        lhsT = yTp[cb][:, tb * P:(tb + 1) * P]
                        nc.tensor.matmul(o_ps[:, 0:512], lhsT, wp[cb][:, 0:512],
                                         start=(cb == 0), stop=(cb == NCB - 1))
                        nc.tensor.matmul(o_ps[:, 512:C], lhsT, wp[cb][:, 512:C],
                                         start=(cb == 0), stop=(cb == NCB - 1))
                    o_sb = o_p.tile([P, C], F32, name="o_sb")
                    nc.vector.tensor_add(o_sb[:], o_ps[:], bp_bc[:])
                    nc.sync.dma_start(out_d[tb * P:(tb + 1) * P, :], o_sb[:])

    # The act-table-load pass assigns each activation the first table set
    # containing its function, which puts Exp in exp_and_others and Ln in
    # natural_log — a 1.3us table reload on every switch. All our functions
    # (Exp, Ln, Identity) live together in natural_log_exp_and_others, so
    # hide the other sets (keeping dict order — act_func_set_id is positional)
    # during this build.
    import concourse.hw_specs as hw_specs
    orig_tables = hw_specs.get_activation_tables

    def _tables(arch, *a, **kw):
        tabs = orig_tables(arch, *a, **kw)
        pref = "natural_log_exp_and_others"
        if pref not in tabs:
            return tabs
        return {k: (v if k == pref else type(v)()) for k, v in tabs.items()}

    import concourse.bacc as bacc_mod
    hw_specs.get_activation_tables = _tables
    bacc_orig = getattr(bacc_mod, "get_activation_tables", None)
    try:
        if bacc_orig is not None:
            bacc_mod.get_activation_tables = _tables
        nc.compile()
    finally:
        hw_specs.get_activation_tables = orig_tables
        if bacc_orig is not None:
            bacc_mod.get_activation_tables = bacc_orig
    return nc


def run(inputs, trace=False):
    import concourse.bass_utils as bass_utils

    nc = _CACHE.get("nc")
    if nc is None:
        nc = _CACHE["nc"] = _build()

    x = np.ascontiguousarray(inputs["x"], dtype=np.float32)
    wa = np.ascontiguousarray(inputs["W_attn"], dtype=np.float32)
    ba = np.ascontiguousarray(inputs["b_attn"], dtype=np.float32)
    wp = np.ascontiguousarray(inputs["W_proj"], dtype=np.float32)
    bp = np.ascontiguousarray(inputs["b_proj"], dtype=np.float32)
    B = x.shape[0]
    in_maps = [
        {"x": np.ascontiguousarray(x[b]), "W_attn": wa, "b_attn": ba,
         "W_proj": wp, "b_proj": bp}
        for b in range(B)
    ]
    res = bass_utils.run_bass_kernel_spmd(
        nc, in_maps, core_ids=list(range(B)), trace=trace)
    out = np.stack([r["out"] for r in res.results], axis=0)
    return out, res


def kernel(**inputs):
    out, _ = run(inputs, trace=False)
    return out


# revision 45
# speedup vs baseline: 1.1233x; 1.0727x over previous
"""Causal self-attention (B=8, T=1024, C=768, H=8 heads) for 8 TRN2 NeuronCores.

Strategy: pure data parallelism — one batch element per core. Each core runs an
identical Bass/Tile program computing the full attention block for its batch
element; weights are replicated. No collectives.

Per-core pipeline (all matmuls in fp32r — full-rate TF32-class PE mode):
  1. x [T,C] -> x^T [C,T] via PE transposes (contraction dim must be on
     partitions).
  2. v = x @ W_v + b_v in [token, feat] layout, stored per 128-token block as
     v_aug [128, 8*97]: per head 96 value columns plus a ones column (the ones
     column makes the P@V matmul also produce the softmax denominator).
  3. Per head h (interleaved so PE work overlaps the ACT exp work of previous
     heads): q^T, k^T = (x @ W_{q,k} + b)^T in [d, token] layout ([96, 1024]),
     then S^T[tk,q] = k^T.T @ q^T per 128-key block over the causally-valid
     query range; P = exp(S*scale) with the diagonal triangle masked by a DVE
     multiply; y_aug^T = sum_tk v_aug^T P (row 96 = softmax denominator since
     P's key-axis is the partition axis); y^T = y_aug^T[0:96] *
     broadcast(Exp(-Ln(denominator))) (Ln+Exp share one ACT table, unlike
     Reciprocal; the broadcast is a 0-stride DMA).
  4. out = y @ W_proj + b_proj with the feature-packed y^T as lhsT.
"""
import sys

sys.path.insert(0, "/opt/trn_rl_repo")

import numpy as np

T, C, H, D = 1024, 768, 8, 96
C3 = 3 * C
P = 128
NT = T // P   # 8 token blocks
NCB = C // P  # 6 feature blocks
DA = D + 1    # 97: head dim + denominator column

_CACHE = {}


def _build():
    import concourse.bacc as bacc
    import concourse.mybir as mybir
    import concourse.tile as tile
    from concourse.masks import make_identity

    F32 = mybir.dt.float32
    F32R = mybir.dt.float32r
    Exp = mybir.ActivationFunctionType.Exp
    Ln = mybir.ActivationFunctionType.Ln
    is_ge = mybir.AluOpType.is_ge
    SCALE = 1.0 / float(np.sqrt(D))

    nc = bacc.Bacc("TRN2", target_bir_lowering=False, debug=False, num_devices=8)
    x_d = nc.dram_tensor("x", [T, C], F32, kind="ExternalInput").ap()
    wa_d = nc.dram_tensor("W_attn", [C, C3], F32, kind="ExternalInput").ap()
    ba_d = nc.dram_tensor("b_attn", [C3], F32, kind="ExternalInput").ap()
    wp_d = nc.dram_tensor("W_proj", [C, C], F32, kind="ExternalInput").ap()
    bp_d = nc.dram_tensor("b_proj", [C], F32, kind="ExternalInput").ap()
    out_d = nc.dram_tensor("out", [T, C], F32, kind="ExternalOutput").ap()

    with tile.TileContext(nc) as tc:
        with tc.tile_pool(name="const", bufs=1) as const_p, \
             tc.tile_pool(name="vp", bufs=1) as v_p, \
             tc.tile_pool(name="qkt", bufs=5) as qk_p, \
             tc.tile_pool(name="yt", bufs=1) as yT_p, \
             tc.tile_pool(name="sm", bufs=2) as sm_p, \
             tc.tile_pool(name="ob", bufs=2) as o_p, \
             tc.tile_pool(name="pp", bufs=4) as p_p, \
             tc.tile_pool(name="ps", bufs=1, space="PSUM") as ps:
            # pack small f32 constants into one 4KB slot:
            #   cols 0:128 identity, 128:256 triangle mask, 256:264 ones8,
            #   264:280 per-head q/k bias columns
            constF = const_p.tile([P, 280], F32, name="constF")
            ident = constF[:, 0:P]
            tri = constF[:, P:2 * P]
            ones8_f = constF[:, 2 * P:2 * P + H]
            b_qk = constF[0:D, 2 * P + H:2 * P + H + 16]
            make_identity(nc, ident)
            # lower-left triangle mask: tri[tk, u] = 1.0 iff u >= tk
            nc.gpsimd.memset(tri, 1.0)
            nc.gpsimd.affine_select(
                out=tri, in_=tri, compare_op=is_ge, fill=0.0,
                base=0, pattern=[[1, P]], channel_multiplier=-1)
            nc.vector.memset(ones8_f, 1.0)
            # col h = q-head h bias, col 8+h = k-head h bias
            nc.sync.dma_start(b_qk, ba_d.rearrange("(a b) -> b a", b=D)[:, 0:16])
            # bias rows broadcast to all partitions via 0-stride DMA
            bv_bc = const_p.tile([P, C], F32, name="bv_bc")
            nc.sync.dma_start(
                bv_bc[:],
                ba_d.unsqueeze(0)[:, 2 * C:3 * C].partition_broadcast(P).squeeze(1))
            vA = [v_p.tile([P, DA * H], F32R, name=f"vA{t}") for t in range(NT)]
            yTp = [yT_p.tile([P, T], F32R, name=f"yTp{cb}") for cb in range(NCB)]
            # DRAM staging for the reciprocal rows (SBUF->SBUF DMA cannot do
            # 0-stride broadcast, DRAM->SBUF can)
            rc_dram = nc.dram_tensor("rc_stage", [2 * H, 512], F32,
                                     kind="Internal").ap()

            with tc.tile_pool(name="xT", bufs=1) as xT_p, \
                 tc.tile_pool(name="wqk", bufs=1) as wqk_p:
                xT = [xT_p.tile([P, T], F32R, name=f"xT{cb}") for cb in range(NCB)]

                # ---- x^T transposes + v projection (scoped W_v / x loads) ----
                with tc.tile_pool(name="xl", bufs=4) as x_p, \
                     tc.tile_pool(name="wv", bufs=1) as wv_p:
                    for jt in range(2):
                        x_ts = []
                        for tb in range(4 * jt, 4 * jt + 4):
                            x_t = x_p.tile([P, C], F32, name="x_t")
                            nc.sync.dma_start(x_t[:], x_d[tb * P:(tb + 1) * P, :])
                            x_ts.append(x_t)
                        for cb in range(NCB):
                            tr_ps = ps.tile([P, 512], F32, name="tr_ps", tag="big", bufs=3)
                            for k in range(4):
                                nc.tensor.transpose(tr_ps[:, k * P:(k + 1) * P],
                                                    x_ts[k][:, cb * P:(cb + 1) * P],
                                                    ident)
                            nc.vector.tensor_copy(xT[cb][:, jt * 512:(jt + 1) * 512],
                                                  tr_ps[:])

                    wv = []
                    for cb in range(NCB):
                        w = wv_p.tile([P, C], F32R, name=f"wv{cb}")
                        nc.sync.dma_start(w[:], wa_d[cb * P:(cb + 1) * P,
                                                     2 * C:3 * C].bitcast(F32R))
                        wv.append(w)
                    for tb in range(NT):
                        v_ps = ps.tile([P, C], F32, name="v_ps", tag="big", bufs=3)
                        for cb in range(NCB):
                            lhsT = xT[cb][:, tb * P:(tb + 1) * P]
                            nc.tensor.matmul(v_ps[:, 0:512], lhsT, wv[cb][:, 0:512],
                                             start=(cb == 0), stop=(cb == NCB - 1))
                            nc.tensor.matmul(v_ps[:, 512:C], lhsT, wv[cb][:, 512:C],
                                             start=(cb == 0), stop=(cb == NCB - 1))
                        for h in range(H):
                            nc.vector.tensor_add(vA[tb][:, DA * h:DA * h + D],
                                                 v_ps[:, D * h:D * h + D],
                                                 bv_bc[:, D * h:D * h + D])
                        # ones columns at local col 96 of each head's group
                        nc.vector.tensor_copy(vA[tb][:, D::DA], ones8_f)

                # ---- per-head: q/k projection + attention, interleaved ----
                wqk = []
                for cb in range(NCB):
                    w = wqk_p.tile([P, 2 * C], F32R, name=f"wqk{cb}")
                    nc.sync.dma_start(w[:], wa_d[cb * P:(cb + 1) * P,
                                                 0:2 * C].bitcast(F32R))
                    wqk.append(w)

                def emit_tail(h, y_sbs):
                    # softmax denominator tail for head h — emitted one head
                    # late so its ACT work (Ln/Exp) doesn't preempt the next
                    # head's critical exp chain
                    for half, y_sb in ((0, y_sbs[0]), (1, y_sbs[1])):
                        q_sl = slice(half * 512, (half + 1) * 512)
                        lnrc = sm_p.tile([1, 1024], F32, name="lnrc", tag="lnrc", bufs=2)
                        ln_s = lnrc[:, 0:512]
                        rc_f = lnrc[:, 512:1024]
                        nc.scalar.activation(ln_s, y_sb[D:DA, :], Ln)
                        nc.scalar.activation(rc_f, ln_s, Exp, scale=-1.0)
                        row = rc_dram[2 * h + half:2 * h + half + 1, :]
                        nc.sync.dma_start(row, rc_f)
                        bc_sb = sm_p.tile([D, 512], F32, name="bc_sb", tag="bcsb", bufs=2)
                        nc.sync.dma_start(bc_sb[:],
                                          row.partition_broadcast(D).squeeze(1))
                        y_n = sm_p.tile([D, 512], F32R, name="y_n", tag="yn", bufs=2)
                        nc.vector.tensor_mul(y_n[:], y_sb[0:D, :], bc_sb[:])
                        # scatter head rows into the feature-packed yT tiles
                        # (partition shift -> must go through DMA)
                        f0 = D * h
                        while f0 < D * (h + 1):
                            cb2, b0 = f0 // P, f0 % P
                            seg = min(P - b0, D * (h + 1) - f0)
                            nc.sync.dma_start(
                                yTp[cb2][b0:b0 + seg, q_sl],
                                y_n[f0 - D * h:f0 - D * h + seg, :])
                            f0 += seg

                pending = None
                for h in range(H):
                    qT = qk_p.tile([D, T], F32R, name="qT", tag="qkt")
                    kT = qk_p.tile([D, T], F32R, name="kT", tag="qkt")
                    for dst, off, bcol in ((qT, D * h, b_qk[:, h:h + 1]),
                                           (kT, C + D * h, b_qk[:, 8 + h:9 + h])):
                        qk_ps = ps.tile([D, T], F32, name="qk_ps", tag="big", bufs=3)
                        for jt in range(2):
                            sl = slice(jt * 512, (jt + 1) * 512)
                            for cb in range(NCB):
                                nc.tensor.matmul(qk_ps[:, sl],
                                                 wqk[cb][:, off:off + D],
                                                 xT[cb][:, sl],
                                                 start=(cb == 0), stop=(cb == NCB - 1))
                            # per-half copyback: overlaps the other half's
                            # matmuls, so the first S block never waits on a
                            # full-width DVE copy
                            nc.vector.tensor_scalar_add(dst[:, sl], qk_ps[:, sl],
                                                        bcol)

                    ptiles = []
                    for ib in range(NT):
                        q0 = P * ib
                        # S^T block over the valid query range [q0, 1024),
                        # split at the 512 PSUM-bank boundary
                        s_ps = ps.tile([P, T], F32, name="s_ps", tag="big", bufs=3)
                        kblk = kT[:, ib * P:(ib + 1) * P]
                        if q0 < 512:
                            nc.tensor.matmul(s_ps[:, q0:512], kblk,
                                             qT[:, q0:512], start=True, stop=True)
                        r0 = max(q0, 512)
                        nc.tensor.matmul(s_ps[:, r0:T], kblk,
                                         qT[:, r0:T], start=True, stop=True)
                        p_t = p_p.tile([P, T], F32R, name="p_t")
                        nc.scalar.activation(p_t[:, q0:T], s_ps[:, q0:T],
                                             Exp, scale=SCALE)
                        # zero the upper triangle of the diagonal 128-col block
                        # (DVE mask-mul: gpsimd's sem wake is too slow here)
                        nc.vector.tensor_mul(p_t[:, q0:q0 + P],
                                             p_t[:, q0:q0 + P], tri)
                        ptiles.append(p_t)
                    # P@V with causal N-restriction; two bank-halves of q,
                    # each its own accumulation group
                    y_l = ps.tile([DA, 512], F32, name="y_l", tag="yps", bufs=2)
                    y_r = ps.tile([DA, 512], F32, name="y_r", tag="yps", bufs=2)
                    for ib in range(NT):
                        q0 = P * ib
                        va = vA[ib][:, DA * h:DA * h + DA]
                        if q0 < 512:
                            nc.tensor.matmul(y_l[:, q0:512], va,
                                             ptiles[ib][:, q0:512],
                                             start=(ib == 0), stop=(ib == 3))
                            nc.tensor.matmul(y_r[:], va, ptiles[ib][:, 512:T],
                                             start=(ib == 0), stop=False)
                        else:
                            nc.tensor.matmul(y_r[:, q0 - 512:512], va,
                                             ptiles[ib][:, q0:T],
                                             start=False, stop=(ib == NT - 1))
                    # stage y to SBUF (frees the PSUM slot quickly); the
                    # denominator tail is emitted one head later
                    y_sbs = []
                    for y_ps in (y_l, y_r):
                        y_sb = sm_p.tile([DA, 512], F32, name="y_sb", tag="ysb", bufs=3)
                        nc.vector.tensor_copy(y_sb[:], y_ps[:])
                        y_sbs.append(y_sb)
                    if pending is not None:
                        emit_tail(*pending)
                    pending = (h, y_sbs)
                if pending is not None:
                    emit_tail(*pending)

            # ---------------- projection ----------------
            with tc.tile_pool(name="wp", bufs=1) as wp_p:
                bp_bc = wp_p.tile([P, C], F32, name="bp_bc", tag="bpbc", bufs=1)
                nc.sync.dma_start(
                    bp_bc[:], bp_d.unsqueeze(0).partition_broadcast(P).squeeze(1))
                wp = []
                for cb in range(NCB):
                    w = wp_p.tile([P, C], F32R, name=f"wp{cb}")
                    nc.sync.dma_start(w[:], wp_d[cb * P:(cb + 1) * P, :].bitcast(F32R))
                    wp.append(w)
                for tb in range(NT):
                    o_ps = ps.tile([P, C], F32, name="o_ps", tag="big", bufs=3)
                    # all 512-wide matmuls first: the [0:512] copyback then
                    # overlaps the 256-wide group, shortening the serial chain
                    for cb in range(NCB):
                        nc.tensor.matmul(o_ps[:, 0:512],
                                         yTp[cb][:, tb * P:(tb + 1) * P],
                                         wp[cb][:, 0:512],
                                         start=(cb == 0), stop=(cb == NCB - 1))
                    o_sb = o_p.tile([P, C], F32, name="o_sb")
                    for cb in range(NCB):
                        nc.tensor.matmul(o_ps[:, 512:C],
                                         yTp[cb][:, tb * P:(tb + 1) * P],
                                         wp[cb][:, 512:C],
                                         start=(cb == 0), stop=(cb == NCB - 1))
                    nc.vector.tensor_add(o_sb[:, 0:512], o_ps[:, 0:512],
                                         bp_bc[:, 0:512])
                    nc.sync.dma_start(out_d[tb * P:(tb + 1) * P, 0:512],
                                      o_sb[:, 0:512])
                    nc.vector.tensor_add(o_sb[:, 512:C], o_ps[:, 512:C],
                                         bp_bc[:, 512:C])
                    nc.sync.dma_start(out_d[tb * P:(tb + 1) * P, 512:C],
                                      o_sb[:, 512:C])

    # The act-table-load pass assigns each activation the first table set
    # containing its function, which puts Exp in exp_and_others and Ln in
    # natural_log — a 1.3us table reload on every switch. All our functions
    # (Exp, Ln, Identity) live together in natural_log_exp_and_others, so
    # hide the other sets (keeping dict order — act_func_set_id is positional)
    # during this build.
    import concourse.hw_specs as hw_specs
    orig_tables = hw_specs.get_activation_tables

    def _tables(arch, *a, **kw):
        tabs = orig_tables(arch, *a, **kw)
        pref = "natural_log_exp_and_others"
        if pref not in tabs:
            return tabs
        return {k: (v if k == pref else type(v)()) for k, v in tabs.items()}

    import concourse.bacc as bacc_mod
    hw_specs.get_activation_tables = _tables
    bacc_orig = getattr(bacc_mod, "get_activation_tables", None)
    try:
        if bacc_orig is not None:
            bacc_mod.get_activation_tables = _tables
        nc.compile()
    finally:
        hw_specs.get_activation_tables = orig_tables
        if bacc_orig is not None:
            bacc_mod.get_activation_tables = bacc_orig
    return nc


def run(inputs, trace=False):
    import concourse.bass_utils as bass_utils

    nc = _CACHE.get("nc")
    if nc is None:
        nc = _CACHE["nc"] = _build()

    x = np.ascontiguousarray(inputs["x"], dtype=np.float32)
    wa = np.ascontiguousarray(inputs["W_attn"], dtype=np.float32)
    ba = np.ascontiguousarray(inputs["b_attn"], dtype=np.float32)
    wp = np.ascontiguousarray(inputs["W_proj"], dtype=np.float32)
    bp = np.ascontiguousarray(inputs["b_proj"], dtype=np.float32)
    B = x.shape[0]
    in_maps = [
        {"x": np.ascontiguousarray(x[b]), "W_attn": wa, "b_attn": ba,
         "W_proj": wp, "b_proj": bp}
        for b in range(B)
    ]
    res = bass_utils.run_bass_kernel_spmd(
        nc, in_maps, core_ids=list(range(B)), trace=trace)
    out = np.stack([r["out"] for r in res.results], axis=0)
    return out, res


def kernel(**inputs):
    out, _ = run(inputs, trace=False)
    return out
